# revision 1
# baseline (speedup 1.0000x reference)
"""Trainium2 Bass kernel for the nn_Decoder LSTM problem — fp8 edition.

Same time-sharded Picard-sweep algorithm as the baseline, with:
  - fp8e4m3 DoubleRow matmuls for the bulk sweeps (2x PE throughput: K=256
    per instruction at the same per-column cost), weights pre-scaled by 16
    to stay in fp8 normal range; activations rescale by 1/16.
  - fp16 polish sweeps at the end (fp8 static weight error floor is ~3.5e-2,
    above the 2e-2 gate) with weights streamed from DRAM (no SBUF residency).
  - biases folded into the x-precompute in phase 1; phase 1 fused with the
    zero-state sweep; batched 8-channel elementwise/activation ops;
    gpsimd offload for muls/casts so the DVE only runs scans.
"""

import sys
import numpy as np
import ml_dtypes

for _p in ("/opt/trn_rl_repo", "/root/.axon_site/_ro/trn_rl_repo"):
    if _p not in sys.path:
        sys.path.insert(0, _p)

import concourse.bass as bass
import concourse.bacc as bacc
import concourse.mybir as mybir
import concourse.tile as tile
from concourse.bass_utils import run_bass_kernel_spmd

D = 1024
T = 8192
KC = 8
DELTA = 48
L = 1024 + DELTA
N_CORES = 8
WS = 16.0          # fp8 weight scale
SCHEDULE = "z88888ff"   # z = fused phase1+zero sweep, 8 = fp8 DR, f = fp16
NI = "12"                # n_inner per sweep (last char repeats)

F16 = mybir.dt.float16
F32 = mybir.dt.float32
F8 = mybir.dt.float8e4
AF = mybir.ActivationFunctionType
ALU = mybir.AluOpType
DR = mybir.MatmulPerfMode.DoubleRow

M_TILES = [(0, 512), (512, 512), (1024, L - 1024)]


def build_nc(schedule=SCHEDULE, ni_str=NI):
    def ni_of(si):
        return int(ni_str[min(si, len(ni_str) - 1)])

    nc = bacc.Bacc(None, target_bir_lowering=False, debug=False)

    # ---- I/O ----
    wh8_d = nc.declare_dram_parameter("wh8", [D, 4 * D], F8, isOutput=False)
    wc8_d = nc.declare_dram_parameter("wc8", [D, D], F8, isOutput=False)
    wh16_d = nc.declare_dram_parameter("wh16", [D, 4 * D], F16, isOutput=False)
    wc16_d = nc.declare_dram_parameter("wc16", [D, D], F16, isOutput=False)
    wx_d = nc.declare_dram_parameter("wx", [D, 3 * D], F16, isOutput=False)
    x_d = nc.declare_dram_parameter("x", [D, L], F16, isOutput=False)
    bias_fio = nc.declare_dram_parameter("bias_fio", [128, 3, KC], F32, isOutput=False)
    bc_d = nc.declare_dram_parameter("bc", [128, KC], F32, isOutput=False)
    tanh_bc_d = nc.declare_dram_parameter("tanh_bc", [128, KC], F32, isOutput=False)
    id16_d = nc.declare_dram_parameter("id16", [128, 128], F16, isOutput=False)
    id1_d = nc.declare_dram_parameter("id1", [128, 128], F16, isOutput=False)
    hb16_d = nc.declare_dram_parameter("hb16", [128, KC, 1], F16, isOutput=False)
    cb16_d = nc.declare_dram_parameter("cb16", [128, KC, 1], F16, isOutput=False)
    hb8_d = nc.declare_dram_parameter("hb8", [128, KC, 1], F8, isOutput=False)
    cb8_d = nc.declare_dram_parameter("cb8", [128, KC, 1], F8, isOutput=False)
    h_out = nc.declare_dram_parameter("h_out", [128, KC, L + 1], F16, isOutput=True)

    pre_dram = nc.dram_tensor("pre_scratch", [3 * KC, 128, L], F16)

    n_sweeps = len(schedule)

    with tile.TileContext(nc) as tc:
        with (
            tc.tile_pool(name="const", bufs=1) as constp,
            tc.tile_pool(name="psum", bufs=4, space="PSUM") as psum,
            tc.tile_pool(name="weights", bufs=1) as wpool,
            tc.tile_pool(name="state", bufs=1) as spool,
            tc.tile_pool(name="gates", bufs=1) as gpool,
            tc.tile_pool(name="work", bufs=1) as wk,
            tc.tile_pool(name="tiny", bufs=2) as tp,
        ):
            bfio_sb = constp.tile([128, 3, KC], F32, tag="bfio")
            bc_sb = constp.tile([128, KC], F32, tag="bc")
            tbc_sb = constp.tile([128, KC], F32, tag="tbc")
            id16_sb = constp.tile([128, 128], F16, tag="id16")
            id1_sb = constp.tile([128, 128], F16, tag="id1")
            nc.sync.dma_start(bfio_sb[...], bias_fio[...])
            nc.sync.dma_start(bc_sb[...], bc_d[...])
            nc.sync.dma_start(tbc_sb[...], tanh_bc_d[...])
            nc.sync.dma_start(id16_sb[...], id16_d[...])
            nc.sync.dma_start(id1_sb[...], id1_d[...])

            wh8_sb = wpool.tile([128, KC, 4 * D], F8, tag="wh8")
            wc8_sb = wpool.tile([128, KC, D], F8, tag="wc8")

            LP = L + 4  # pad so fp8 channel stride is 4-byte aligned
            Hf = spool.tile([128, KC, LP], F16, tag="H")
            Cf = spool.tile([128, KC, LP], F16, tag="C")
            H8 = spool.tile([128, KC, LP], F8, tag="H8")
            C8 = spool.tile([128, KC, LP], F8, tag="C8")
            nc.sync.dma_start(Hf[:, :, 0:1], hb16_d[...])
            nc.sync.dma_start(Cf[:, :, 0:1], cb16_d[...])
            nc.sync.dma_start(H8[:, :, 0:1], hb8_d[...])
            nc.sync.dma_start(C8[:, :, 0:1], cb8_d[...])

            f_all = gpool.tile([128, KC, 512], F16, tag="f")
            i_all = gpool.tile([128, KC, 512], F16, tag="i")
            o_all = gpool.tile([128, KC, 512], F16, tag="o")

            # deferred boundary flush state (python-side closure)
            state = {"deferred": None}

            def make_flush(t_b, fl, bl, ol, write16, write8):
                def flush():
                    cprev = Cf[:, :, t_b - 1:t_b]
                    tb1 = tp.tile([128, KC, 1], F16, tag="tb1")
                    nc.gpsimd.tensor_tensor(tb1[...], fl[...], cprev, ALU.mult)
                    nc.gpsimd.tensor_tensor(Cf[:, :, t_b:t_b + 1], tb1[...],
                                            bl[...], ALU.add)
                    if write8:
                        nc.gpsimd.tensor_copy(C8[:, :, t_b:t_b + 1],
                                              Cf[:, :, t_b:t_b + 1])
                    tcb = tp.tile([128, KC, 1], F16, tag="tcb")
                    nc.scalar.activation(tcb[...], Cf[:, :, t_b:t_b + 1], AF.Tanh)
                    if write16:
                        nc.gpsimd.tensor_tensor(Hf[:, :, t_b:t_b + 1], ol[...],
                                                tcb[...], ALU.mult)
                    if write8:
                        hb8t = tp.tile([128, KC, 1], F16, tag="hb8t")
                        nc.gpsimd.tensor_tensor(hb8t[...], ol[...], tcb[...],
                                                ALU.mult)
                        nc.gpsimd.tensor_copy(H8[:, :, t_b:t_b + 1], hb8t[...])
                return flush

            def stage_boundary(t0, N, bb, write16, write8):
                fl = tp.tile([128, KC, 1], F16, tag="fl")
                bl = tp.tile([128, KC, 1], F16, tag="bl")
                ol = tp.tile([128, KC, 1], F16, tag="ol")
                nc.gpsimd.tensor_copy(fl[...], f_all[:, :, N - 1:N])
                nc.gpsimd.tensor_copy(bl[...], bb[:, :, N - 1:N])
                nc.gpsimd.tensor_copy(ol[...], o_all[:, :, N - 1:N])
                state["deferred"] = make_flush(t0 + N, fl, bl, ol, write16, write8)

            def scan_ch(t0, N, bb, ch):
                nc.vector.tensor_tensor_scan(
                    Cf[:, ch, t0 + 1:t0 + N],
                    f_all[:, ch, :N - 1],
                    bb[:, ch, :N - 1],
                    Cf[:, ch, t0:t0 + 1],
                    ALU.mult, ALU.add,
                )

            def do_scans(t0, N, bb, write8):
                for ch in range(KC):
                    scan_ch(t0, N, bb, ch)
                if write8:
                    cast_c8(t0, N)

            def cast_c8(t0, N):
                nc.vector.tensor_copy(C8[:, :, t0 + 1:t0 + N],
                                      Cf[:, :, t0 + 1:t0 + N])

            def do_h(t0, N, write16, write8):
                tch = wk.tile([128, KC, 512], F16, tag="ct", bufs=2)
                nc.scalar.activation(tch[:, :, :N - 1], Cf[:, :, t0 + 1:t0 + N],
                                     AF.Tanh)
                if write16:
                    nc.vector.tensor_tensor(Hf[:, :, t0 + 1:t0 + N],
                                            o_all[:, :, :N - 1],
                                            tch[:, :, :N - 1], ALU.mult)
                if write8:
                    nc.gpsimd.tensor_tensor(H8[:, :, t0 + 1:t0 + N],
                                            o_all[:, :, :N - 1],
                                            tch[:, :, :N - 1], ALU.mult)

            # ================= phase 1 + zero sweep =================
            ni_z = ni_of(0)
            with tc.tile_pool(name="ph1", bufs=2) as ph1:
                # queue recurrent weight loads first (they are large; overlap
                # with phase-1 compute which only needs xT/wx slabs)
                nc.sync.dma_start(
                    wh8_sb[...], wh8_d[:, :].rearrange("(c p) m -> p c m", p=128))
                nc.sync.dma_start(
                    wc8_sb[...], wc8_d[:, :].rearrange("(c p) m -> p c m", p=128))
                for (t0, N) in M_TILES:
                    xT = ph1.tile([128, KC, 512], F16, tag="xT")
                    nc.sync.dma_start(
                        xT[:, :, :N],
                        x_d[:, t0:t0 + N].rearrange("(c p) t -> p c t", p=128))
                    for g, garr in enumerate((f_all, i_all, o_all)):
                        for half in range(2):
                            wxs = ph1.tile([128, KC, 512], F16, tag="wxs")
                            nc.sync.dma_start(
                                wxs[...],
                                wx_d[:, g * D + half * 512: g * D + (half + 1) * 512]
                                .rearrange("(c p) m -> p c m", p=128))
                            for gq in range(2):   # pair of gd
                                ps = psum.tile([128, 2, 512], F32, tag="pp")
                                for j in range(2):
                                    gd = half * 4 + gq * 2 + j
                                    col = (gq * 2 + j) * 128
                                    for kc in range(KC):
                                        nc.tensor.matmul(
                                            ps[:, j, :N],
                                            wxs[:, kc, col:col + 128],
                                            xT[:, kc, :N],
                                            start=(kc == 0), stop=(kc == KC - 1))
                                for j in range(2):
                                    gd = half * 4 + gq * 2 + j
                                    pre_t = ph1.tile([128, 512], F16, tag="pre_t")
                                    nc.scalar.activation(
                                        pre_t[:, :N], ps[:, j, :N], AF.Identity,
                                        bias=bfio_sb[:, g, gd:gd + 1])
                                    nc.sync.dma_start(
                                        pre_dram[g * KC + gd, :, t0:t0 + N],
                                        pre_t[:, :N])
                                    nc.scalar.activation(
                                        garr[:, gd, :N], ps[:, j, :N], AF.Sigmoid,
                                        bias=bfio_sb[:, g, gd:gd + 1])
                    # ---- zero-sweep tail for this m-tile ----
                    bb = wk.tile([128, KC, 512], F16, tag="bb")
                    for ch in range(KC):
                        nc.vector.tensor_scalar(
                            bb[:, ch, :N], i_all[:, ch, :N],
                            tbc_sb[:, ch:ch + 1], None, ALU.mult)
                    if state["deferred"] is not None:
                        state["deferred"]()
                        state["deferred"] = None
                    for ch in range(KC):
                        scan_ch(t0, N, bb, ch)
                    for r in range(ni_z):
                        ct = wk.tile([128, KC, 512], F16, tag="ct", bufs=2)
                        bb = wk.tile([128, KC, 512], F16, tag="bb")
                        for cq in range(4):
                            ps = psum.tile([128, 2, 512], F32, tag="pp")
                            for j in range(2):
                                ch = cq * 2 + j
                                for kc in range(KC):
                                    nc.tensor.matmul(
                                        ps[:, j, :N],
                                        wc8_sb[:, kc, ch * 128:(ch + 1) * 128],
                                        Cf[:, kc, t0:t0 + N],
                                        start=(kc == 0), stop=(kc == KC - 1))
                                nc.scalar.activation(
                                    ct[:, ch, :N], ps[:, j, :N], AF.Tanh,
                                    bias=bc_sb[:, ch:ch + 1], scale=1.0 / WS)
                                nc.gpsimd.tensor_tensor(
                                    bb[:, ch, :N], i_all[:, ch, :N],
                                    ct[:, ch, :N], ALU.mult)
                                scan_ch(t0, N, bb, ch)
                    cast_c8(t0, N)
                    do_h(t0, N, write16=False, write8=True)
                    stage_boundary(t0, N, bb, write16=True, write8=True)
                state["deferred"]()
                state["deferred"] = None

            # ================= main sweeps =================
            with (
                tc.tile_pool(name="prestream", bufs=2) as prepool,
                tc.tile_pool(name="fstream", bufs=3) as fpool,
            ):
                def gate_group_8(garr, g, t0, N, pre_tiles):
                    """fp8 DR gate (f/i/o): wh8[:, :, g*D + ch*128...] @ H8."""
                    for cq in range(4):
                        ps = psum.tile([128, 2, 512], F32, tag="pp")
                        for j in range(2):
                            ch = cq * 2 + j
                            col = g * D + ch * 128
                            for kc in range(0, KC, 2):
                                nc.tensor.matmul(
                                    ps[:, j, :N],
                                    wh8_sb[:, kc:kc + 2, col:col + 128],
                                    H8[:, kc:kc + 2, t0:t0 + N],
                                    start=(kc == 0), stop=False, perf_mode=DR)
                            nc.tensor.matmul(
                                ps[:, j, :N], id16_sb[:, :],
                                pre_tiles[ch][:, :N], start=False, stop=True)
                        nc.scalar.activation(
                            garr[:, cq * 2:cq * 2 + 2, :N], ps[:, :, :N],
                            AF.Sigmoid, scale=1.0 / WS)

                def psA_part(zA, t0, N):
                    # ct h-part: wh8[:,:,3D..] @ H8 -> zA (with bias), DR
                    for cq in range(4):
                        psA = psum.tile([128, 2, 512], F32, tag="pp")
                        for j in range(2):
                            ch = cq * 2 + j
                            col = 3 * D + ch * 128
                            for kc in range(0, KC, 2):
                                nc.tensor.matmul(
                                    psA[:, j, :N],
                                    wh8_sb[:, kc:kc + 2, col:col + 128],
                                    H8[:, kc:kc + 2, t0:t0 + N],
                                    start=(kc == 0), stop=(kc == KC - 2),
                                    perf_mode=DR)
                        for j in range(2):
                            ch = cq * 2 + j
                            nc.scalar.activation(
                                zA[:, ch, :N], psA[:, j, :N], AF.Identity,
                                bias=bc_sb[:, ch:ch + 1], scale=1.0 / WS)

                def refine_8(ct, zA, t0, N):
                    # initial c-solve preact: zA + wc8 @ C8(prev sweep), DR
                    for cq in range(4):
                        ps = psum.tile([128, 2, 512], F32, tag="pp")
                        for j in range(2):
                            ch = cq * 2 + j
                            nc.tensor.matmul(
                                ps[:, j, :N], id16_sb[:, :], zA[:, ch, :N],
                                start=True, stop=False)
                            for kc in range(0, KC, 2):
                                nc.tensor.matmul(
                                    ps[:, j, :N],
                                    wc8_sb[:, kc:kc + 2, ch * 128:(ch + 1) * 128],
                                    C8[:, kc:kc + 2, t0:t0 + N],
                                    start=False, stop=(kc == KC - 2),
                                    perf_mode=DR)
                        nc.scalar.activation(
                            ct[:, cq * 2:cq * 2 + 2, :N], ps[:, :, :N],
                            AF.Tanh, scale=1.0 / WS)

                def refine_mixed(ct, bb, zA, t0, N, scale, wcc=None):
                    """Pipelined refinement solve: per-ch [mm -> act -> bb -> scan].
                    c-part reads fresh Cf16 (mixed dtype when fp8 weights)."""
                    for cq in range(4):
                        ps = psum.tile([128, 2, 512], F32, tag="pp")
                        for j in range(2):
                            ch = cq * 2 + j
                            nc.tensor.matmul(
                                ps[:, j, :N],
                                id16_sb[:, :] if scale != 1.0 else id1_sb[:, :],
                                zA[:, ch, :N], start=True, stop=False)
                            if wcc is None:
                                for kc in range(KC):
                                    nc.tensor.matmul(
                                        ps[:, j, :N],
                                        wc8_sb[:, kc, ch * 128:(ch + 1) * 128],
                                        Cf[:, kc, t0:t0 + N],
                                        start=False, stop=(kc == KC - 1))
                            else:
                                wcs = wcc[ch // 4]
                                ccol = (ch % 4) * 128
                                for kc in range(KC):
                                    nc.tensor.matmul(
                                        ps[:, j, :N],
                                        wcs[:, kc, ccol:ccol + 128],
                                        Cf[:, kc, t0:t0 + N],
                                        start=False, stop=(kc == KC - 1))
                            nc.scalar.activation(
                                ct[:, ch, :N], ps[:, j, :N], AF.Tanh, scale=scale)
                            nc.gpsimd.tensor_tensor(
                                bb[:, ch, :N], i_all[:, ch, :N],
                                ct[:, ch, :N], ALU.mult)
                            scan_ch(t0, N, bb, ch)

                def sweep_8(ni, last, next_is_8):
                    assert ni > 0
                    for ti, (t0, N) in enumerate(M_TILES):
                        pf, pi, po = [], [], []
                        for g, lst in ((0, pf), (1, pi), (2, po)):
                            for gd in range(KC):
                                pt = prepool.tile([128, 512], F16,
                                                  tag=f"p{g}")
                                nc.sync.dma_start(
                                    pt[:, :N], pre_dram[g * KC + gd, :, t0:t0 + N])
                                lst.append(pt)
                        if state.get("zA_pipe") is not None:
                            zA = state["zA_pipe"]
                            state["zA_pipe"] = None
                        else:
                            zA = wk.tile([128, KC, 512], F16, tag="zA", bufs=2)
                            psA_part(zA, t0, N)
                        if state.get("ct_pipe") is not None:
                            ct = state["ct_pipe"]
                            state["ct_pipe"] = None
                        else:
                            ct = wk.tile([128, KC, 512], F16, tag="ct", bufs=2)
                            refine_8(ct, zA, t0, N)
                        gate_group_8(f_all, 0, t0, N, pf)
                        gate_group_8(i_all, 1, t0, N, pi)
                        if state["deferred"] is not None:
                            state["deferred"]()
                            state["deferred"] = None
                        bb = wk.tile([128, KC, 512], F16, tag="bb")
                        for ch in range(KC):
                            nc.gpsimd.tensor_tensor(
                                bb[:, ch, :N], i_all[:, ch, :N],
                                ct[:, ch, :N], ALU.mult)
                            scan_ch(t0, N, bb, ch)
                        gate_group_8(o_all, 2, t0, N, po)
                        nxt = None
                        if ti + 1 < len(M_TILES):
                            nxt = M_TILES[ti + 1]
                        elif next_is_8:
                            nxt = M_TILES[0]
                        for r in range(ni):
                            ct2 = wk.tile([128, KC, 512], F16, tag="ct", bufs=2)
                            bb = wk.tile([128, KC, 512], F16, tag="bb")
                            refine_mixed(ct2, bb, zA, t0, N, 1.0 / WS)
                            # pipeline next tile's ct h-part into the scan gap
                            if r == 0 and nxt is not None:
                                zA2 = wk.tile([128, KC, 512], F16, tag="zA",
                                              bufs=2)
                                psA_part(zA2, nxt[0], nxt[1])
                                state["zA_pipe"] = zA2
                        if nxt is not None and state.get("zA_pipe") is not None:
                            ct_n = wk.tile([128, KC, 512], F16, tag="ct",
                                           bufs=2)
                            refine_8(ct_n, state["zA_pipe"], nxt[0], nxt[1])
                            state["ct_pipe"] = ct_n
                        cast_c8(t0, N)
                        do_h(t0, N, write16=last, write8=True)
                        stage_boundary(t0, N, bb, write16=True, write8=True)
                    state["deferred"]()
                    state["deferred"] = None

                def slab(tag="slab"):
                    return fpool.tile([128, KC, 512], F16, tag=tag, name="wslab")

                def load_slab(s, src, c0):
                    nc.sync.dma_start(
                        s[...], src[:, c0:c0 + 512]
                        .rearrange("(c p) m -> p c m", p=128))

                def gate_group_16(garr, g, t0, N, pre_tiles):
                    for half in range(2):
                        s = slab()
                        load_slab(s, wh16_d, g * D + half * 512)
                        for cq in range(2):
                            ps = psum.tile([128, 2, 512], F32, tag="pp")
                            for j in range(2):
                                ch = half * 4 + cq * 2 + j
                                col = (cq * 2 + j) * 128
                                for kc in range(KC):
                                    nc.tensor.matmul(
                                        ps[:, j, :N],
                                        s[:, kc, col:col + 128],
                                        Hf[:, kc, t0:t0 + N],
                                        start=(kc == 0), stop=False)
                                nc.tensor.matmul(
                                    ps[:, j, :N], id1_sb[:, :],
                                    pre_tiles[ch][:, :N],
                                    start=False, stop=True)
                            ch0 = half * 4 + cq * 2
                            nc.scalar.activation(
                                garr[:, ch0:ch0 + 2, :N], ps[:, :, :N],
                                AF.Sigmoid)

                def sweep_16(ni):
                    for (t0, N) in M_TILES:
                        pf, pi, po = [], [], []
                        for g, lst in ((0, pf), (1, pi), (2, po)):
                            for gd in range(KC):
                                pt = prepool.tile([128, 512], F16, tag=f"p{g}")
                                nc.sync.dma_start(
                                    pt[:, :N], pre_dram[g * KC + gd, :, t0:t0 + N])
                                lst.append(pt)
                        ct = wk.tile([128, KC, 512], F16, tag="ct", bufs=2)
                        zA = wk.tile([128, KC, 512], F16, tag="zA", bufs=2)
                        wcc = [fpool.tile([128, KC, 512], F16, tag="wcc", bufs=2,
                                          name=f"wcc{_k}")
                               for _k in range(2)]
                        load_slab(wcc[0], wc16_d, 0)
                        load_slab(wcc[1], wc16_d, 512)

                        def refine_16(ct_dst):
                            for cq in range(4):
                                ps = psum.tile([128, 2, 512], F32, tag="pp")
                                for j in range(2):
                                    ch = cq * 2 + j
                                    nc.tensor.matmul(
                                        ps[:, j, :N], id1_sb[:, :],
                                        zA[:, ch, :N], start=True, stop=False)
                                    wcs = wcc[ch // 4]
                                    ccol = (ch % 4) * 128
                                    for kc in range(KC):
                                        nc.tensor.matmul(
                                            ps[:, j, :N],
                                            wcs[:, kc, ccol:ccol + 128],
                                            Cf[:, kc, t0:t0 + N],
                                            start=False, stop=(kc == KC - 1))
                                for j in range(2):
                                    ch = cq * 2 + j
                                    nc.scalar.activation(
                                        ct_dst[:, ch, :N], ps[:, j, :N], AF.Tanh)

                        if ni > 0:
                            # psA = whC @ Hf -> zA (with bias), then c-part
                            for half in range(2):
                                s = slab()
                                load_slab(s, wh16_d, 3 * D + half * 512)
                                for cq in range(2):
                                    psA = psum.tile([128, 2, 512], F32, tag="pp")
                                    for j in range(2):
                                        ch = half * 4 + cq * 2 + j
                                        col = (cq * 2 + j) * 128
                                        for kc in range(KC):
                                            nc.tensor.matmul(
                                                psA[:, j, :N],
                                                s[:, kc, col:col + 128],
                                                Hf[:, kc, t0:t0 + N],
                                                start=(kc == 0),
                                                stop=(kc == KC - 1))
                                    for j in range(2):
                                        ch = half * 4 + cq * 2 + j
                                        nc.scalar.activation(
                                            zA[:, ch, :N], psA[:, j, :N],
                                            AF.Identity, bias=bc_sb[:, ch:ch + 1])
                            refine_16(ct)
                        else:
                            for half in range(2):
                                s = slab()
                                load_slab(s, wh16_d, 3 * D + half * 512)
                                for cq in range(2):
                                    ps = psum.tile([128, 2, 512], F32, tag="pp")
                                    for j in range(2):
                                        ch = half * 4 + cq * 2 + j
                                        col = (cq * 2 + j) * 128
                                        for kc in range(KC):
                                            nc.tensor.matmul(
                                                ps[:, j, :N],
                                                s[:, kc, col:col + 128],
                                                Hf[:, kc, t0:t0 + N],
                                                start=(kc == 0), stop=False)
                                        wcs = wcc[ch // 4]
                                        ccol = (ch % 4) * 128
                                        for kc in range(KC):
                                            nc.tensor.matmul(
                                                ps[:, j, :N],
                                                wcs[:, kc, ccol:ccol + 128],
                                                Cf[:, kc, t0:t0 + N],
                                                start=False,
                                                stop=(kc == KC - 1))
                                    for j in range(2):
                                        ch = half * 4 + cq * 2 + j
                                        nc.scalar.activation(
                                            ct[:, ch, :N], ps[:, j, :N],
                                            AF.Tanh, bias=bc_sb[:, ch:ch + 1])
                        gate_group_16(f_all, 0, t0, N, pf)
                        gate_group_16(i_all, 1, t0, N, pi)
                        if state["deferred"] is not None:
                            state["deferred"]()
                            state["deferred"] = None
                        bb = wk.tile([128, KC, 512], F16, tag="bb")
                        for ch in range(KC):
                            nc.gpsimd.tensor_tensor(
                                bb[:, ch, :N], i_all[:, ch, :N],
                                ct[:, ch, :N], ALU.mult)
                            scan_ch(t0, N, bb, ch)
                        gate_group_16(o_all, 2, t0, N, po)
                        for r in range(ni):
                            ct2 = wk.tile([128, KC, 512], F16, tag="ct", bufs=2)
                            bb = wk.tile([128, KC, 512], F16, tag="bb")
                            refine_mixed(ct2, bb, zA, t0, N, 1.0, wcc=wcc)
                        do_h(t0, N, write16=True, write8=False)
                        stage_boundary(t0, N, bb, write16=True, write8=False)
                    state["deferred"]()
                    state["deferred"] = None

                n8 = sum(1 for c in schedule if c == "8")
                seen8 = 0
                for si, mode in enumerate(schedule):
                    if mode == "z":
                        continue  # already emitted (fused with phase 1)
                    elif mode == "8":
                        seen8 += 1
                        sweep_8(ni_of(si), last=(seen8 == n8),
                                next_is_8=(seen8 < n8))
                    elif mode == "f":
                        sweep_16(ni_of(si))

                nc.sync.dma_start(h_out[...], Hf[:, :, :L + 1])

    nc.compile()
    return nc


# ------------------------- host side -------------------------

def _q8(a):
    return (np.asarray(a, np.float32) * WS).astype(ml_dtypes.float8_e4m3)


def _prep_core_inputs(inputs):
    x = np.asarray(inputs["target_seq"], np.float32)
    W_f = np.asarray(inputs["W_f"], np.float32)
    W_i = np.asarray(inputs["W_i"], np.float32)
    W_C = np.asarray(inputs["W_C"], np.float32)
    W_o = np.asarray(inputs["W_o"], np.float32)

    wh16 = np.concatenate(
        [W_f[:, :D].T, W_i[:, :D].T, W_o[:, :D].T, W_C[:, :D].T], axis=1
    ).astype(np.float16)                      # [D, 4D] cols = [f|i|o|C]
    wc16 = np.ascontiguousarray(W_C[:, D:].T).astype(np.float16)
    wh8 = _q8(wh16)
    wc8 = _q8(wc16)
    wx = np.concatenate(
        [W_f[:, D:].T, W_i[:, D:].T, W_o[:, D:].T], axis=1
    ).astype(np.float16)                      # [D, 3D]

    def vec_pc(v):
        return np.ascontiguousarray(np.asarray(v, np.float32).reshape(KC, 128).T)

    bias_fio = np.stack([vec_pc(inputs["b_f"]), vec_pc(inputs["b_i"]),
                         vec_pc(inputs["b_o"])], axis=1)  # [128, 3, 8]
    bc = vec_pc(inputs["b_C"])
    tanh_bc = np.tanh(bc).astype(np.float32)
    id16 = (np.eye(128) * WS).astype(np.float16)
    id1 = np.eye(128, dtype=np.float16)

    h0 = np.asarray(inputs["encoder_h"], np.float32)
    c0 = np.asarray(inputs["encoder_c"], np.float32)

    in_maps = []
    for core in range(N_CORES):
        if core == 0:
            rows = slice(0, L)
            hb = vec_pc(h0)[:, :, None]
            cb = vec_pc(c0)[:, :, None]
        else:
            rows = slice(1024 * core - DELTA, 1024 * core + 1024)
            hb = np.zeros((128, KC, 1), np.float32)
            cb = np.zeros((128, KC, 1), np.float32)
        x_chunk = np.ascontiguousarray(x[rows].T).astype(np.float16)
        in_maps.append({
            "wh8": wh8, "wc8": wc8, "wh16": wh16, "wc16": wc16, "wx": wx,
            "x": x_chunk,
            "bias_fio": bias_fio.astype(np.float32), "bc": bc.astype(np.float32),
            "tanh_bc": tanh_bc,
            "id16": id16, "id1": id1,
            "hb16": hb.astype(np.float16), "cb16": cb.astype(np.float16),
            "hb8": hb.astype(ml_dtypes.float8_e4m3),
            "cb8": cb.astype(ml_dtypes.float8_e4m3),
        })
    return in_maps


def _gather_output(results):
    out = np.empty((T, D), np.float32)
    for core in range(N_CORES):
        h = np.asarray(results[core]["h_out"]).reshape(128, KC, L + 1)
        chunk = np.transpose(h, (2, 1, 0)).reshape(L + 1, D).astype(np.float32)
        if core == 0:
            out[0:1024] = chunk[1:1025]
        else:
            out[1024 * core:1024 * (core + 1)] = chunk[DELTA + 1:L + 1]
    return out


_NC_CACHE = {}


def _get_nc(schedule=SCHEDULE, ni=NI):
    key = (schedule, ni)
    if key not in _NC_CACHE:
        _NC_CACHE[key] = build_nc(schedule, ni)
    return _NC_CACHE[key]


def kernel(**inputs) -> np.ndarray:
    nc = _get_nc()
    in_maps = _prep_core_inputs(inputs)
    res = run_bass_kernel_spmd(nc, in_maps, list(range(N_CORES)))
    return _gather_output(res.results)


if __name__ == "__main__":
    nc = build_nc()
    print("built ok")



# revision 11
# speedup vs baseline: 1.0413x; 1.0413x over previous
"""Trainium2 Bass kernel for the nn_Decoder LSTM problem — pipelined edition.

Same time-sharded Picard-sweep algorithm as the baseline (8 cores x 1072-step
chunks, fp8 DoubleRow bulk sweeps + fp16 polish, exact DVE scan for the linear
c recurrence), restructured for PE occupancy:

  - jobs = (sweep, tile) pairs; per-tile gate buffers (tags bufs=3/3/2) so the
    next job's gate/psA/ct0 matmuls are emitted into the current job's
    scan-ladder windows (software pipelining across tiles AND sweeps).
  - boundary flush runs directly on the vector engine reading the live gate
    buffers (no gpsimd copy chain, no deferral machinery).
  - recurrent-weight DMA queued after the first x/wx slabs so phase 1 starts
    immediately.
"""

import os
import sys
import numpy as np
import ml_dtypes

for _p in ("/opt/trn_rl_repo", "/root/.axon_site/_ro/trn_rl_repo"):
    if _p not in sys.path:
        sys.path.insert(0, _p)

import concourse.bass as bass
import concourse.bacc as bacc
import concourse.mybir as mybir
import concourse.tile as tile
from concourse.bass_utils import run_bass_kernel_spmd
from contextlib import ExitStack

D = 1024
T = 8192
KC = 8
DELTA = 48
L = 1024 + DELTA
N_CORES = 8
WS = 16.0          # fp8 weight scale
SCHEDULE = os.environ.get("LSTM_SCHED", "z88888ff")
NI = os.environ.get("LSTM_NI", "12")   # n_inner per sweep (last char repeats)

F16 = mybir.dt.float16
F32 = mybir.dt.float32
F8 = mybir.dt.float8e4
AF = mybir.ActivationFunctionType
ALU = mybir.AluOpType
DR = mybir.MatmulPerfMode.DoubleRow

M_TILES = [(0, 512), (512, 512), (1024, L - 1024)]


def build_nc(schedule=SCHEDULE, ni_str=NI):
    def ni_of(si):
        return int(ni_str[min(si, len(ni_str) - 1)])

    nc = bacc.Bacc(None, target_bir_lowering=False, debug=False)

    # ---- I/O ----
    wh8_d = nc.declare_dram_parameter("wh8", [D, 4 * D], F8, isOutput=False)
    wc8_d = nc.declare_dram_parameter("wc8", [D, D], F8, isOutput=False)
    wh16_d = nc.declare_dram_parameter("wh16", [D, 4 * D], F16, isOutput=False)
    wc16_d = nc.declare_dram_parameter("wc16", [D, D], F16, isOutput=False)
    wx_d = nc.declare_dram_parameter("wx", [D, 3 * D], F16, isOutput=False)
    x_d = nc.declare_dram_parameter("x", [D, L], F16, isOutput=False)
    bias_fio = nc.declare_dram_parameter("bias_fio", [128, 3, KC], F32, isOutput=False)
    bc_d = nc.declare_dram_parameter("bc", [128, KC], F32, isOutput=False)
    tanh_bc_d = nc.declare_dram_parameter("tanh_bc", [128, KC], F32, isOutput=False)
    id16_d = nc.declare_dram_parameter("id16", [128, 128], F16, isOutput=False)
    id1_d = nc.declare_dram_parameter("id1", [128, 128], F16, isOutput=False)
    hb16_d = nc.declare_dram_parameter("hb16", [128, KC, 1], F16, isOutput=False)
    cb16_d = nc.declare_dram_parameter("cb16", [128, KC, 1], F16, isOutput=False)
    hb8_d = nc.declare_dram_parameter("hb8", [128, KC, 1], F8, isOutput=False)
    cb8_d = nc.declare_dram_parameter("cb8", [128, KC, 1], F8, isOutput=False)
    h_out = nc.declare_dram_parameter("h_out", [128, KC, L + 1], F16, isOutput=True)

    pre_dram = nc.dram_tensor("pre_scratch", [3 * KC, 128, L], F16)

    # sweep descriptors: schedule[0] must be 'z' (fused with phase 1)
    assert schedule[0] == "z"
    sweeps = [(si, m, ni_of(si)) for si, m in enumerate(schedule)]
    last8_si = max((si for si, m, _ in sweeps if m == "8"), default=None)

    with tile.TileContext(nc) as tc:
        with (
            tc.tile_pool(name="const", bufs=1) as constp,
            tc.tile_pool(name="psum", bufs=4, space="PSUM") as psum,
            tc.tile_pool(name="state", bufs=1) as spool,
            tc.tile_pool(name="gates", bufs=1) as gpool,
            tc.tile_pool(name="work", bufs=1) as wk,
            tc.tile_pool(name="prestream", bufs=2) as prepool,
            tc.tile_pool(name="tiny", bufs=2) as tp,
        ):
            _nmc = [0]

            def _nm(p):
                _nmc[0] += 1
                return f"{p}{_nmc[0]}"

            wstack = ExitStack()
            fstack = ExitStack()
            p16 = {}   # filled with fpool/wcc pool at the 8->f boundary

            bfio_sb = constp.tile([128, 3, KC], F32, tag="bfio")
            bc_sb = constp.tile([128, KC], F32, tag="bc")
            tbc_sb = constp.tile([128, KC], F32, tag="tbc")
            id16_sb = constp.tile([128, 128], F16, tag="id16")
            id1_sb = constp.tile([128, 128], F16, tag="id1")
            nc.sync.dma_start(bfio_sb[...], bias_fio[...])
            nc.sync.dma_start(bc_sb[...], bc_d[...])
            nc.sync.dma_start(tbc_sb[...], tanh_bc_d[...])
            nc.sync.dma_start(id16_sb[...], id16_d[...])
            nc.sync.dma_start(id1_sb[...], id1_d[...])

            wc8_sb = constp.tile([128, KC, D], F8, tag="wc8")
            nc.sync.dma_start(
                wc8_sb[...], wc8_d[:, :].rearrange("(c p) m -> p c m", p=128))
            wh8_ref = {}   # wh8_ref["sb"] set when the wh8 pool opens

            LP = L + 4
            Hf = spool.tile([128, KC, LP], F16, tag="H")
            Cf = spool.tile([128, KC, LP], F16, tag="C")
            H8 = spool.tile([128, KC, LP], F8, tag="H8")
            C8 = spool.tile([128, KC, LP], F8, tag="C8")
            nc.sync.dma_start(Hf[:, :, 0:1], hb16_d[...])
            nc.sync.dma_start(Cf[:, :, 0:1], cb16_d[...])
            nc.sync.dma_start(H8[:, :, 0:1], hb8_d[...])
            nc.sync.dma_start(C8[:, :, 0:1], cb8_d[...])

            # ------------- shared helpers -------------
            def do_scan(t0, N, fgt, bb, ch):
                nc.vector.tensor_tensor_scan(
                    Cf[:, ch, t0 + 1:t0 + N],
                    fgt[:, ch, :N - 1],
                    bb[:, ch, :N - 1],
                    Cf[:, ch, t0:t0 + 1],
                    ALU.mult, ALU.add,
                )

            def gate8(garr, g, t0, N, pre_tiles):
                for cq in range(4):
                    ps = psum.tile([128, 2, 512], F32, tag="pp")
                    for j in range(2):
                        ch = cq * 2 + j
                        col = g * D + ch * 128
                        for kc in range(0, KC, 2):
                            nc.tensor.matmul(
                                ps[:, j, :N],
                                wh8_ref["sb"][:, kc:kc + 2, col:col + 128],
                                H8[:, kc:kc + 2, t0:t0 + N],
                                start=(kc == 0), stop=False, perf_mode=DR)
                        nc.tensor.matmul(
                            ps[:, j, :N], id16_sb[:, :],
                            pre_tiles[ch][:, :N], start=False, stop=True)
                    nc.scalar.activation(
                        garr[:, cq * 2:cq * 2 + 2, :N], ps[:, :, :N],
                        AF.Sigmoid, scale=1.0 / WS)

            def psA8(zA, t0, N):
                for cq in range(4):
                    psA = psum.tile([128, 2, 512], F32, tag="pp")
                    for j in range(2):
                        ch = cq * 2 + j
                        col = 3 * D + ch * 128
                        for kc in range(0, KC, 2):
                            nc.tensor.matmul(
                                psA[:, j, :N],
                                wh8_ref["sb"][:, kc:kc + 2, col:col + 128],
                                H8[:, kc:kc + 2, t0:t0 + N],
                                start=(kc == 0), stop=(kc == KC - 2),
                                perf_mode=DR)
                    for j in range(2):
                        ch = cq * 2 + j
                        nc.scalar.activation(
                            zA[:, ch, :N], psA[:, j, :N], AF.Identity,
                            bias=bc_sb[:, ch:ch + 1], scale=1.0 / WS)

            def ct0_8(ct, zA, t0, N):
                for cq in range(4):
                    ps = psum.tile([128, 2, 512], F32, tag="pp")
                    for j in range(2):
                        ch = cq * 2 + j
                        nc.tensor.matmul(
                            ps[:, j, :N], id16_sb[:, :], zA[:, ch, :N],
                            start=True, stop=False)
                        for kc in range(0, KC, 2):
                            nc.tensor.matmul(
                                ps[:, j, :N],
                                wc8_sb[:, kc:kc + 2, ch * 128:(ch + 1) * 128],
                                C8[:, kc:kc + 2, t0:t0 + N],
                                start=False, stop=(kc == KC - 2),
                                perf_mode=DR)
                    nc.scalar.activation(
                        ct[:, cq * 2:cq * 2 + 2, :N], ps[:, :, :N],
                        AF.Tanh, scale=1.0 / WS)

            def slab16(c0, n512=512):
                s = p16["fpool"].tile([128, KC, 512], F16, tag="slab")
                nc.sync.dma_start(
                    s[...], wh16_d[:, c0:c0 + 512]
                    .rearrange("(c p) m -> p c m", p=128))
                return s

            def gate16(garr, g, t0, N, pre_tiles):
                for half in range(2):
                    s = slab16(g * D + half * 512)
                    for cq in range(2):
                        ps = psum.tile([128, 2, 512], F32, tag="pp")
                        for j in range(2):
                            ch = half * 4 + cq * 2 + j
                            col = (cq * 2 + j) * 128
                            for kc in range(KC):
                                nc.tensor.matmul(
                                    ps[:, j, :N],
                                    s[:, kc, col:col + 128],
                                    Hf[:, kc, t0:t0 + N],
                                    start=(kc == 0), stop=False)
                            nc.tensor.matmul(
                                ps[:, j, :N], id1_sb[:, :],
                                pre_tiles[ch][:, :N],
                                start=False, stop=True)
                        ch0 = half * 4 + cq * 2
                        nc.scalar.activation(
                            garr[:, ch0:ch0 + 2, :N], ps[:, :, :N],
                            AF.Sigmoid)

            def psA16(zA, t0, N):
                for half in range(2):
                    s = slab16(3 * D + half * 512)
                    for cq in range(2):
                        psA = psum.tile([128, 2, 512], F32, tag="pp")
                        for j in range(2):
                            ch = half * 4 + cq * 2 + j
                            col = (cq * 2 + j) * 128
                            for kc in range(KC):
                                nc.tensor.matmul(
                                    psA[:, j, :N],
                                    s[:, kc, col:col + 128],
                                    Hf[:, kc, t0:t0 + N],
                                    start=(kc == 0), stop=(kc == KC - 1))
                        for j in range(2):
                            ch = half * 4 + cq * 2 + j
                            nc.scalar.activation(
                                zA[:, ch, :N], psA[:, j, :N],
                                AF.Identity, bias=bc_sb[:, ch:ch + 1])

            def ct0_16(ct, zA, t0, N, wcc):
                for cq in range(4):
                    ps = psum.tile([128, 2, 512], F32, tag="pp")
                    for j in range(2):
                        ch = cq * 2 + j
                        nc.tensor.matmul(
                            ps[:, j, :N], id1_sb[:, :],
                            zA[:, ch, :N], start=True, stop=False)
                        wcs = wcc[ch // 4]
                        ccol = (ch % 4) * 128
                        for kc in range(KC):
                            nc.tensor.matmul(
                                ps[:, j, :N],
                                wcs[:, kc, ccol:ccol + 128],
                                Cf[:, kc, t0:t0 + N],
                                start=False, stop=(kc == KC - 1))
                    for j in range(2):
                        ch = cq * 2 + j
                        nc.scalar.activation(
                            ct[:, ch, :N], ps[:, j, :N], AF.Tanh)

            def load_pre(g, t0, N):
                lst = []
                for gd in range(KC):
                    pt = prepool.tile([128, 512], F16, tag=f"p{g}", bufs=2)
                    nc.sync.dma_start(
                        pt[:, :N], pre_dram[g * KC + gd, :, t0:t0 + N])
                    lst.append(pt)
                return lst

            # ------------- job emitters -------------
            def emit_fi(jb):
                t0, N, mode = jb["t0"], jb["N"], jb["mode"]
                jb["f"] = gpool.tile([128, KC, N], F16, tag="fg", bufs=3, name=_nm("fg"))
                jb["i"] = gpool.tile([128, KC, N], F16, tag="ig", bufs=3, name=_nm("ig"))
                pf = load_pre(0, t0, N)
                pi = load_pre(1, t0, N)
                if mode == "8":
                    gate8(jb["f"], 0, t0, N, pf)
                    gate8(jb["i"], 1, t0, N, pi)
                else:
                    gate16(jb["f"], 0, t0, N, pf)
                    gate16(jb["i"], 1, t0, N, pi)

            def emit_opsA(jb):
                t0, N, mode = jb["t0"], jb["N"], jb["mode"]
                jb["o"] = gpool.tile([128, KC, N], F16, tag="og", bufs=2, name=_nm("og"))
                jb["zA"] = wk.tile([128, KC, N], F16, tag="zA", bufs=2, name=_nm("zA"))
                po = load_pre(2, t0, N)
                if mode == "8":
                    gate8(jb["o"], 2, t0, N, po)
                    psA8(jb["zA"], t0, N)
                else:
                    gate16(jb["o"], 2, t0, N, po)
                    psA16(jb["zA"], t0, N)

            def emit_ct0(jb):
                t0, N, mode = jb["t0"], jb["N"], jb["mode"]
                ct = wk.tile([128, KC, N], F16, tag="ct", bufs=2)
                jb["ct0"] = ct
                if mode == "8":
                    ct0_8(ct, jb["zA"], t0, N)
                else:
                    wcc = [p16["wccp"].tile([128, KC, 512], F16, tag="wcc",
                                            bufs=2, name=f"wcc{_k}")
                           for _k in range(2)]
                    for _k in range(2):
                        nc.sync.dma_start(
                            wcc[_k][...], wc16_d[:, _k * 512:_k * 512 + 512]
                            .rearrange("(c p) m -> p c m", p=128))
                    jb["wcc"] = wcc
                    ct0_16(ct, jb["zA"], t0, N, wcc)

            def emit_initial_ladder(jb):
                t0, N = jb["t0"], jb["N"]
                bb = wk.tile([128, KC, N], F16, tag="bb", bufs=1)
                for ch in range(KC):
                    nc.gpsimd.tensor_tensor(
                        bb[:, ch, :N], jb["i"][:, ch, :N],
                        jb["ct0"][:, ch, :N], ALU.mult)
                    do_scan(t0, N, jb["f"], bb, ch)
                jb["bb"] = bb

            def emit_refine(jb):
                t0, N, mode = jb["t0"], jb["N"], jb["mode"]
                ct = wk.tile([128, KC, N], F16, tag="ct", bufs=2)
                bb = wk.tile([128, KC, N], F16, tag="bb", bufs=1)
                scale = (1.0 / WS) if mode == "8" else 1.0
                for cq in range(4):
                    ps = psum.tile([128, 2, 512], F32, tag="pp")
                    for j in range(2):
                        ch = cq * 2 + j
                        nc.tensor.matmul(
                            ps[:, j, :N],
                            id16_sb[:, :] if mode == "8" else id1_sb[:, :],
                            jb["zA"][:, ch, :N], start=True, stop=False)
                        if mode == "8":
                            for kc in range(KC):
                                nc.tensor.matmul(
                                    ps[:, j, :N],
                                    wc8_sb[:, kc, ch * 128:(ch + 1) * 128],
                                    Cf[:, kc, t0:t0 + N],
                                    start=False, stop=(kc == KC - 1))
                        else:
                            wcs = jb["wcc"][ch // 4]
                            ccol = (ch % 4) * 128
                            for kc in range(KC):
                                nc.tensor.matmul(
                                    ps[:, j, :N],
                                    wcs[:, kc, ccol:ccol + 128],
                                    Cf[:, kc, t0:t0 + N],
                                    start=False, stop=(kc == KC - 1))
                        nc.scalar.activation(
                            ct[:, ch, :N], ps[:, j, :N], AF.Tanh, scale=scale)
                        nc.gpsimd.tensor_tensor(
                            bb[:, ch, :N], jb["i"][:, ch, :N],
                            ct[:, ch, :N], ALU.mult)
                        do_scan(t0, N, jb["f"], bb, ch)
                jb["bb"] = bb

            def emit_tail(jb):
                t0, N = jb["t0"], jb["N"]
                w16, w8 = jb["write16"], jb["write8"]
                t_b = t0 + N
                if w8:
                    nc.vector.tensor_copy(C8[:, :, t0 + 1:t0 + N],
                                          Cf[:, :, t0 + 1:t0 + N])
                tch = wk.tile([128, KC, N], F16, tag="ct", bufs=2)
                nc.scalar.activation(tch[:, :, :N - 1], Cf[:, :, t0 + 1:t0 + N],
                                     AF.Tanh)
                if w16:
                    nc.vector.tensor_tensor(Hf[:, :, t0 + 1:t0 + N],
                                            jb["o"][:, :, :N - 1],
                                            tch[:, :, :N - 1], ALU.mult)
                if w8:
                    nc.gpsimd.tensor_tensor(H8[:, :, t0 + 1:t0 + N],
                                            jb["o"][:, :, :N - 1],
                                            tch[:, :, :N - 1], ALU.mult)
                # boundary flush (vector engine, reads live gate buffers)
                tb1 = tp.tile([128, KC, 1], F16, tag="tb1")
                nc.vector.tensor_tensor(tb1[...], jb["f"][:, :, N - 1:N],
                                        Cf[:, :, t_b - 1:t_b], ALU.mult)
                nc.vector.tensor_tensor(Cf[:, :, t_b:t_b + 1], tb1[...],
                                        jb["bb"][:, :, N - 1:N], ALU.add)
                if w8:
                    nc.gpsimd.tensor_copy(C8[:, :, t_b:t_b + 1],
                                          Cf[:, :, t_b:t_b + 1])
                tcb = tp.tile([128, KC, 1], F16, tag="tcb")
                nc.scalar.activation(tcb[...], Cf[:, :, t_b:t_b + 1], AF.Tanh)
                nc.vector.tensor_tensor(Hf[:, :, t_b:t_b + 1],
                                        jb["o"][:, :, N - 1:N],
                                        tcb[...], ALU.mult)
                if w8:
                    nc.gpsimd.tensor_copy(H8[:, :, t_b:t_b + 1],
                                          Hf[:, :, t_b:t_b + 1])

            # ---- job list for the pipelined main sweeps ----
            jobs = []
            for si, mode, ni in sweeps[1:]:
                for ti, (t0, N) in enumerate(M_TILES):
                    w16 = (mode == "f") or (si == last8_si) or (si == len(sweeps) - 1)
                    w8 = (mode == "8") and (si != last8_si)
                    jobs.append({"si": si, "mode": mode, "ni": ni, "ti": ti,
                                 "t0": t0, "N": N,
                                 "write16": w16, "write8": w8})
            K = len(jobs)

            def open_f_pools():
                if "fpool" not in p16:
                    wstack.close()   # free fp8 weight residency
                    p16["fpool"] = fstack.enter_context(
                        tc.tile_pool(name="fstream", bufs=2))
                    p16["wccp"] = fstack.enter_context(
                        tc.tile_pool(name="wccp", bufs=1))

            def boundary(k):
                # True if job k is fp16 but the previous job is fp8/absent
                return (k < K and jobs[k]["mode"] == "f"
                        and (k == 0 or jobs[k - 1]["mode"] == "8"))

            def prime(k):
                if boundary(k):
                    open_f_pools()
                emit_fi(jobs[k])
                emit_opsA(jobs[k])
                emit_ct0(jobs[k])
                if k + 1 < K and not boundary(k + 1):
                    emit_fi(jobs[k + 1])

            # ================= phase 1 + fused zero sweep =================
            ni_z = sweeps[0][2]
            zjobs = [{"t0": t0, "N": N, "mode": "z",
                      "write16": False, "write8": True}
                     for (t0, N) in M_TILES]
            with tc.tile_pool(name="ph1", bufs=2) as ph1:
                for ti, (t0, N) in enumerate(M_TILES):
                    jb = zjobs[ti]
                    xT = ph1.tile([128, KC, 512], F16, tag="xT")
                    nc.sync.dma_start(
                        xT[:, :, :N],
                        x_d[:, t0:t0 + N].rearrange("(c p) t -> p c t", p=128))
                    jb["f"] = gpool.tile([128, KC, N], F16, tag="fg", bufs=3, name=_nm("fg"))
                    jb["i"] = gpool.tile([128, KC, N], F16, tag="ig", bufs=3, name=_nm("ig"))
                    jb["o"] = gpool.tile([128, KC, N], F16, tag="og", bufs=2, name=_nm("og"))
                    for g, garr in enumerate((jb["f"], jb["i"], jb["o"])):
                        for half in range(2):
                            wxs = ph1.tile([128, KC, 512], F16, tag="wxs")
                            nc.sync.dma_start(
                                wxs[...],
                                wx_d[:, g * D + half * 512: g * D + (half + 1) * 512]
                                .rearrange("(c p) m -> p c m", p=128))
                            for gq in range(2):
                                ps = psum.tile([128, 2, 512], F32, tag="pp")
                                for j in range(2):
                                    gd = half * 4 + gq * 2 + j
                                    col = (gq * 2 + j) * 128
                                    for kc in range(KC):
                                        nc.tensor.matmul(
                                            ps[:, j, :N],
                                            wxs[:, kc, col:col + 128],
                                            xT[:, kc, :N],
                                            start=(kc == 0), stop=(kc == KC - 1))
                                for j in range(2):
                                    gd = half * 4 + gq * 2 + j
                                    pre_t = ph1.tile([128, 512], F16, tag="pre_t")
                                    nc.scalar.activation(
                                        pre_t[:, :N], ps[:, j, :N], AF.Identity,
                                        bias=bfio_sb[:, g, gd:gd + 1])
                                    nc.sync.dma_start(
                                        pre_dram[g * KC + gd, :, t0:t0 + N],
                                        pre_t[:, :N])
                                    nc.scalar.activation(
                                        garr[:, gd, :N], ps[:, j, :N], AF.Sigmoid,
                                        bias=bfio_sb[:, g, gd:gd + 1])
                    # ---- zero-sweep ladder for this m-tile ----
                    bb = wk.tile([128, KC, N], F16, tag="bb", bufs=1)
                    for ch in range(KC):
                        nc.vector.tensor_scalar(
                            bb[:, ch, :N], jb["i"][:, ch, :N],
                            tbc_sb[:, ch:ch + 1], None, ALU.mult)
                        do_scan(t0, N, jb["f"], bb, ch)
                    jb["bb"] = bb
                    for r in range(ni_z):
                        ct = wk.tile([128, KC, N], F16, tag="ct", bufs=2)
                        bb = wk.tile([128, KC, N], F16, tag="bb", bufs=1)
                        for cq in range(4):
                            ps = psum.tile([128, 2, 512], F32, tag="pp")
                            for j in range(2):
                                ch = cq * 2 + j
                                for kc in range(KC):
                                    nc.tensor.matmul(
                                        ps[:, j, :N],
                                        wc8_sb[:, kc, ch * 128:(ch + 1) * 128],
                                        Cf[:, kc, t0:t0 + N],
                                        start=(kc == 0), stop=(kc == KC - 1))
                                nc.scalar.activation(
                                    ct[:, ch, :N], ps[:, j, :N], AF.Tanh,
                                    bias=bc_sb[:, ch:ch + 1], scale=1.0 / WS)
                                nc.gpsimd.tensor_tensor(
                                    bb[:, ch, :N], jb["i"][:, ch, :N],
                                    ct[:, ch, :N], ALU.mult)
                                do_scan(t0, N, jb["f"], bb, ch)
                        jb["bb"] = bb
                    emit_tail(jb)

            # ================= pipelined main sweeps =================
            if K > 0 and jobs[0]["mode"] == "8":
                wpool = wstack.enter_context(tc.tile_pool(name="weights", bufs=1))
                wh8_ref["sb"] = wpool.tile([128, KC, 4 * D], F8, tag="wh8", name="wh8sb")
                nc.sync.dma_start(
                    wh8_ref["sb"][...],
                    wh8_d[:, :].rearrange("(c p) m -> p c m", p=128))
            if K > 0:
                prime(0)
            k = 0
            while k < K:
                jb = jobs[k]
                emit_initial_ladder(jb)
                fillers = []
                nxt = k + 1
                if nxt < K and not boundary(nxt):
                    fillers = [lambda: emit_opsA(jobs[nxt]),
                               lambda: emit_ct0(jobs[nxt])]
                for r in range(jb["ni"]):
                    emit_refine(jb)
                    if fillers:
                        fillers.pop(0)()
                for fn in fillers:
                    fn()
                emit_tail(jb)
                if nxt < K and boundary(nxt):
                    # pipeline restart across the 8->f boundary
                    prime(nxt)
                elif k + 2 < K and not boundary(k + 2):
                    emit_fi(jobs[k + 2])
                elif k + 2 < K and boundary(k + 2):
                    pass  # handled by prime at the boundary
                k += 1

            nc.sync.dma_start(h_out[...], Hf[:, :, :L + 1])
            wstack.close()
            fstack.close()

    nc.compile()
    return nc


# ------------------------- host side -------------------------

def _q8(a):
    return (np.asarray(a, np.float32) * WS).astype(ml_dtypes.float8_e4m3)


def _prep_core_inputs(inputs):
    x = np.asarray(inputs["target_seq"], np.float32)
    W_f = np.asarray(inputs["W_f"], np.float32)
    W_i = np.asarray(inputs["W_i"], np.float32)
    W_C = np.asarray(inputs["W_C"], np.float32)
    W_o = np.asarray(inputs["W_o"], np.float32)

    wh16 = np.concatenate(
        [W_f[:, :D].T, W_i[:, :D].T, W_o[:, :D].T, W_C[:, :D].T], axis=1
    ).astype(np.float16)                      # [D, 4D] cols = [f|i|o|C]
    wc16 = np.ascontiguousarray(W_C[:, D:].T).astype(np.float16)
    wh8 = _q8(wh16)
    wc8 = _q8(wc16)
    wx = np.concatenate(
        [W_f[:, D:].T, W_i[:, D:].T, W_o[:, D:].T], axis=1
    ).astype(np.float16)                      # [D, 3D]

    def vec_pc(v):
        return np.ascontiguousarray(np.asarray(v, np.float32).reshape(KC, 128).T)

    bias_fio = np.stack([vec_pc(inputs["b_f"]), vec_pc(inputs["b_i"]),
                         vec_pc(inputs["b_o"])], axis=1)  # [128, 3, 8]
    bc = vec_pc(inputs["b_C"])
    tanh_bc = np.tanh(bc).astype(np.float32)
    id16 = (np.eye(128) * WS).astype(np.float16)
    id1 = np.eye(128, dtype=np.float16)

    h0 = np.asarray(inputs["encoder_h"], np.float32)
    c0 = np.asarray(inputs["encoder_c"], np.float32)

    in_maps = []
    for core in range(N_CORES):
        if core == 0:
            rows = slice(0, L)
            hb = vec_pc(h0)[:, :, None]
            cb = vec_pc(c0)[:, :, None]
        else:
            rows = slice(1024 * core - DELTA, 1024 * core + 1024)
            hb = np.zeros((128, KC, 1), np.float32)
            cb = np.zeros((128, KC, 1), np.float32)
        x_chunk = np.ascontiguousarray(x[rows].T).astype(np.float16)
        in_maps.append({
            "wh8": wh8, "wc8": wc8, "wh16": wh16, "wc16": wc16, "wx": wx,
            "x": x_chunk,
            "bias_fio": bias_fio.astype(np.float32), "bc": bc.astype(np.float32),
            "tanh_bc": tanh_bc,
            "id16": id16, "id1": id1,
            "hb16": hb.astype(np.float16), "cb16": cb.astype(np.float16),
            "hb8": hb.astype(ml_dtypes.float8_e4m3),
            "cb8": cb.astype(ml_dtypes.float8_e4m3),
        })
    return in_maps


def _gather_output(results):
    out = np.empty((T, D), np.float32)
    for core in range(N_CORES):
        h = np.asarray(results[core]["h_out"]).reshape(128, KC, L + 1)
        chunk = np.transpose(h, (2, 1, 0)).reshape(L + 1, D).astype(np.float32)
        if core == 0:
            out[0:1024] = chunk[1:1025]
        else:
            out[1024 * core:1024 * (core + 1)] = chunk[DELTA + 1:L + 1]
    return out


_NC_CACHE = {}


def _get_nc(schedule=SCHEDULE, ni=NI):
    key = (schedule, ni)
    if key not in _NC_CACHE:
        _NC_CACHE[key] = build_nc(schedule, ni)
    return _NC_CACHE[key]


def kernel(**inputs) -> np.ndarray:
    nc = _get_nc()
    in_maps = _prep_core_inputs(inputs)
    res = run_bass_kernel_spmd(nc, in_maps, list(range(N_CORES)))
    return _gather_output(res.results)


if __name__ == "__main__":
    nc = build_nc()
    print("built ok")
